# revision 1
# baseline (speedup 1.0000x reference)
"""Trainium2 Bass kernel for nn_AttentionBlock (GroupNorm + 1x1-conv QKV
self-attention + proj + residual).

Full input x: [16, 256, 32, 32] f32.  Sharding: data-parallel over batch,
2 batch items per core across 8 NeuronCores.  Each core runs the same SPMD
program on its own batch shard; no collectives.

Per-batch math (C=256, N=1024 positions):
  h   = GroupNorm(8 groups)(x) * nw + nb
  q   = (Wq h + bq) * C^-0.5          [c, i]   (scale folded into Wq/bq host-side)
  k   = Wk h + bk                     [c, j]
  vT  = (Wv h)^T                      [j, c]   (v bias folded into proj bias host-side)
  ST  = k^T q                         [j, i]   (S transposed -> contraction dims stay on partitions)
  E   = exp(ST)                       (no max subtraction; logits are O(1) by construction)
  Out = v E = sum_j vT[j,c] E[j,i]    [c, i]   (unnormalized)
  rs  = ones^T E                      [1, i]   (softmax denominator)
  P   = Wp (Out * (1/rs))             [o, i]   (normalization commutes through proj)
  y   = x + P + pb_eff

All matmuls run in bf16 (fp32 PSUM accumulation); x, stats and the residual
path stay fp32.  rstd = exp(-0.5*ln(var+eps)) keeps ScalarE on a single
activation table (natural_log_exp_and_others: Ln/Exp/Square/Identity).
"""

import functools
import sys

import numpy as np

sys.path.insert(0, "/opt/trn_rl_repo")

import ml_dtypes

BF16 = ml_dtypes.bfloat16

B, C, H, W = 16, 256, 32, 32
N = H * W            # 1024 positions
NCORES = 8
BLOC = B // NCORES   # 2 batch items per core
CT = C // 128        # 2 channel tiles
JT = N // 128        # 8 position tiles (partition-side)
NCH = N // 512       # 2 free-dim chunks of 512
GROUPS = 8
GPT = GROUPS // CT   # 4 groups per 128-channel tile
GSIZE = (C // GROUPS) * N  # elements per group = 32*1024
EPS = 1e-5


@functools.lru_cache(maxsize=1)
def _build():
    from contextlib import ExitStack

    import concourse.bacc as bacc
    import concourse.mybir as mybir
    import concourse.tile as tile

    f32 = mybir.dt.float32
    bf16 = mybir.dt.bfloat16
    fp8 = mybir.dt.float8e4
    Alu = mybir.AluOpType
    Act = mybir.ActivationFunctionType
    Ax = mybir.AxisListType

    # The act-table insertion pass greedily picks the first table containing
    # each function, thrashing between exp_and_others and natural_log (5 table
    # loads, 1.28us each).  Every activation we use (Square, Ln, Exp,
    # Identity, Copy, MemsetZero) lives in natural_log_exp_and_others, so
    # blank out every other candidate (keeping list order => act_func_set_id
    # indices stay valid for walrus) to get exactly one load.
    if not getattr(bacc, "_act_tables_patched", False):
        _orig_get_tables = bacc.get_activation_tables

        def _only_ln_exp(arch):
            return {
                name: (funcs if name == "natural_log_exp_and_others" else set())
                for name, funcs in _orig_get_tables(arch).items()
            }

        bacc.get_activation_tables = _only_ln_exp
        bacc._act_tables_patched = True

    nc = bacc.Bacc("TRN2", target_bir_lowering=False)

    x_d = nc.dram_tensor("x", [BLOC, C, N], f32, kind="ExternalInput")
    # packed weights: per 128-channel tile, [wqT | wkT | wvT | wpT] side by side
    wpk_d = nc.dram_tensor("wpack", [C, 4 * C], bf16, kind="ExternalInput")
    # packed per-channel vectors: [bq, bk, nw, nb, pb]
    vpk_d = nc.dram_tensor("vpack", [C, 5], f32, kind="ExternalInput")
    bd_d = nc.dram_tensor("blockdiag", [128, GPT], f32, kind="ExternalInput")
    eb_d = nc.dram_tensor("ebcast", [GPT, 128], f32, kind="ExternalInput")
    y_d = nc.dram_tensor("y", [BLOC, C, N], f32, kind="ExternalOutput")

    with tile.TileContext(nc) as tc, ExitStack() as stack:
        cp = stack.enter_context(tc.tile_pool(name="consts", bufs=1))
        sp2 = stack.enter_context(tc.tile_pool(name="sbuf2", bufs=2))
        sp4 = stack.enter_context(tc.tile_pool(name="sbuf4", bufs=4))
        sp16 = stack.enter_context(tc.tile_pool(name="sbuf16", bufs=16))
        ppb = stack.enter_context(tc.tile_pool(name="psumb", bufs=3, space="PSUM"))
        pps = stack.enter_context(tc.tile_pool(name="psums", bufs=2, space="PSUM"))

        # --- first batch's x loads go out before anything else (the GN stats
        # chain is the critical path; weights aren't needed until qkv) ---
        xs_first = []
        for t in range(CT):
            xt = sp4.tile([128, N], f32, tag="x")
            nc.gpsimd.dma_start(xt[:], x_d[0, 128 * t : 128 * (t + 1), :])
            xs_first.append(xt)

        # --- constants: 6 DMAs total (on ScalarE queue), ones/eps via memset.
        # Tiny GN constants (bd/eb/vpack) go FIRST: the DMA engines drain in
        # arrival order and the groupnorm stats matmuls need bd/eb within the
        # first ~5us, while the big weight pack isn't read until qkv. ---
        bd = cp.tile([128, GPT], f32, tag="bd")
        nc.scalar.dma_start(bd[:], bd_d[:])
        eb = cp.tile([GPT, 128], f32, tag="eb")
        nc.scalar.dma_start(eb[:], eb_d[:])
        vpk = []
        for t in range(CT):
            vt_ = cp.tile([128, 5], f32, tag=f"vpk{t}")
            nc.scalar.dma_start(vt_[:], vpk_d[128 * t : 128 * (t + 1), :])
            vpk.append(vt_)
        wpk = []
        for t in range(CT):
            wt = cp.tile([128, 4 * C], bf16, tag=f"wpk{t}")
            nc.scalar.dma_start(wt[:], wpk_d[128 * t : 128 * (t + 1), :])
            wpk.append(wt)

        def wslice(t, which, m):  # lhsT tile [128c, 128o]
            off = which * C + 128 * m
            return wpk[t][:, off : off + 128]

        def wv_full(t):  # rhs [128c, 256o] for the vT matmul
            return wpk[t][:, 2 * C : 3 * C]

        bq = [vpk[t][:, 0:1] for t in range(CT)]
        bk = [vpk[t][:, 1:2] for t in range(CT)]
        nw = [vpk[t][:, 2:3] for t in range(CT)]
        nb = [vpk[t][:, 3:4] for t in range(CT)]
        pb = [vpk[t][:, 4:5] for t in range(CT)]

        # fp8 ones for the DoubleRow rowsum reduction: pair elements must sit
        # at a byte step that is a multiple of 16, so lay them out in a
        # [128, 32] tile and slice [p, 2(step16), 1]
        o16 = cp.tile([128, 32], fp8, tag="o16")
        nc.vector.memset(o16[:], 1.0)
        o16r = o16[:].rearrange("p (i n) -> p i n", i=2)
        epsc = cp.tile([GPT, 1], f32, tag="eps")
        nc.vector.memset(epsc[:], EPS)

        for b in range(BLOC):
            # ---------------- load x (GpSimd DMA queue) ----------------
            if b == 0:
                xs = xs_first
            else:
                # later batches load on the ScalarE DMA queue, BEHIND the
                # weight pack: their stats don't start until the previous
                # batch's drains clear anyway, and this keeps the x transfers
                # from jumping ahead of the weights in the DMA engines.
                xs = []
                for t in range(CT):
                    xt = sp4.tile([128, N], f32, tag="x")
                    nc.scalar.dma_start(xt[:], x_d[b, 128 * t : 128 * (t + 1), :])
                    xs.append(xt)

            # ---------------- groupnorm stats + normalize (per-tile chains) ----
            # blockdiag is pre-scaled by 1/GSIZE on host, so the cross-partition
            # matmul emits [mean, E[x^2]] directly.  Later batches' stats get
            # boosted priority so DVE/ACT run them as soon as x lands instead
            # of finishing the previous batch's (slack-rich) drains first.
            import contextlib as _cl

            prio = tc.high_priority() if b > 0 else _cl.nullcontext()
            prio.__enter__()
            hs = []
            for t in range(CT):
                stat2 = sp4.tile([128, 2], f32, tag="stat2")
                sqs = sp4.tile([128, N], bf16, tag="sqscratch")
                nc.vector.tensor_reduce(stat2[:, 0:1], xs[t][:], Ax.X, Alu.add)
                nc.scalar.activation(sqs[:], xs[t][:], Act.Square, accum_out=stat2[:, 1:2])
                gps = pps.tile([GPT, 2], f32, tag="small")
                nc.tensor.matmul(gps[:], bd[:], stat2[:], start=True, stop=True)
                statb = sp4.tile([GPT, 2], f32, tag="statb")  # [mean, rstd]
                nc.vector.tensor_copy(statb[:, 0:1], gps[:, 0:1])
                msq = sp4.tile([GPT, 2], f32, tag="msq")  # [mean^2, var]
                nc.vector.tensor_mul(msq[:, 0:1], statb[:, 0:1], statb[:, 0:1])
                nc.vector.tensor_sub(msq[:, 1:2], gps[:, 1:2], msq[:, 0:1])
                # rstd = exp(-0.5*ln(var+eps)) -- stays on the Ln/Exp/Square table
                lnv = sp4.tile([GPT, 1], f32, tag="lnv")
                nc.scalar.activation(lnv[:], msq[:, 1:2], Act.Ln, bias=epsc[:])
                nc.scalar.activation(statb[:, 1:2], lnv[:], Act.Exp, scale=-0.5)

                bc = pps.tile([128, 2], f32, tag="small")  # [mean_c, rstd_c]
                nc.tensor.matmul(bc[:], eb[:], statb[:], start=True, stop=True)
                ab = sp4.tile([128, 2], f32, tag="ab")  # [A, B]
                nc.vector.tensor_mul(ab[:, 0:1], bc[:, 1:2], nw[t])
                t1 = sp4.tile([128, 1], f32, tag="t1")
                nc.vector.tensor_mul(t1[:], bc[:, 0:1], ab[:, 0:1])
                nc.vector.tensor_sub(ab[:, 1:2], nb[t], t1[:])
                # h = x*A + B, split by column chunk across DVE and ACT so the
                # first qkv matmuls (which read h[:, 0:512]) start sooner
                ht = sp4.tile([128, N], bf16, tag="h")
                nc.vector.tensor_scalar(
                    ht[:, 0:512], xs[t][:, 0:512], ab[:, 0:1], ab[:, 1:2], Alu.mult, Alu.add
                )
                nc.gpsimd.tensor_scalar(
                    ht[:, 512:1024], xs[t][:, 512:1024], ab[:, 0:1], ab[:, 1:2],
                    Alu.mult, Alu.add,
                )
                hs.append(ht)

            # ---------------- qkv ----------------
            # q/k land in single [128, 2N] fp8 tiles: both 128-channel halves
            # side by side, which is exactly the DoubleRow pair layout
            # [p, 2(step N), n] for a 256-deep contraction in one matmul.
            qkt = []
            for wi, b_, wn in ((0, bq, "q"), (1, bk, "k")):
                ot = sp4.tile([128, 2 * N], fp8, tag=f"qk_{wn}")
                # ch-major order: the first ST matmuls only read the ch0
                # halves of q and k, so draining both m-tiles' ch0 chunks
                # first unblocks the attention phase earlier
                for ch in range(NCH):
                    for m in range(CT):
                        ps = pps.tile([128, 512], f32, tag="small")
                        for t in range(CT):
                            nc.tensor.matmul(
                                ps[:], wslice(t, wi, m),
                                hs[t][:, 512 * ch : 512 * (ch + 1)],
                                start=(t == 0), stop=(t == CT - 1),
                            )
                        # bias-add copy: q drains on DVE, k on ScalarE
                        dst = ot[:, N * m + 512 * ch : N * m + 512 * (ch + 1)]
                        if wn == "q":
                            nc.vector.tensor_scalar_add(dst, ps[:], b_[m])
                        else:
                            nc.scalar.activation(dst, ps[:], Act.Identity, bias=b_[m])
                qkt.append(ot[:].rearrange("p (i n) -> p i n", i=2))
            qr, kr = qkt
            prio.__exit__(None, None, None)

            # vT in j-pair tiles [128, 2, C]
            vtp = []
            for u in range(JT // 2):
                ps = pps.tile([128, 2 * C], f32, tag="small")  # one bank, 2 j's
                for r in range(2):
                    j = 2 * u + r
                    for t in range(CT):
                        nc.tensor.matmul(
                            ps[:, C * r : C * (r + 1)],
                            hs[t][:, 128 * j : 128 * (j + 1)], wv_full(t),
                            start=(t == 0), stop=(t == CT - 1),
                        )
                vt = sp16.tile([128, 2 * C], fp8, tag="vt")
                nc.vector.tensor_copy(vt[:], ps[:])
                vtp.append(vt)
            vtr = [v[:].rearrange("p (i n) -> p i n", i=2) for v in vtp]

            # ---------------- ST = k^T q (DoubleRow, 256-deep) ; E = exp(ST/16) ----
            # The softmax scale C^-0.5 = 1/16 is applied by the Exp activation
            # (func(in*scale)), so q/k keep full magnitude for fp8 range.
            estp = []
            for j in range(JT):
                if j % 2 == 0:
                    est = sp16.tile([128, 2 * N], fp8, tag="est")
                    estp.append(est)
                ps = ppb.tile([128, N], f32, tag="big")  # 2 banks, 1 per chunk
                # last j-tile: exp per chunk so the softmax-denominator chain
                # (which gates the whole normalize->proj->store tail) starts
                # as soon as its first 512 columns are ready
                expchunks = NCH if j == JT - 1 else 1
                for ch in range(NCH):
                    nc.tensor.matmul(
                        ps[:, 512 * ch : 512 * (ch + 1)],
                        kr[:, :, 128 * j : 128 * (j + 1)],
                        qr[:, :, 512 * ch : 512 * (ch + 1)],
                        start=True, stop=True,
                        perf_mode=mybir.MatmulPerfMode.DoubleRow,
                    )
                w_ = N // expchunks
                for e in range(expchunks):
                    nc.scalar.activation(
                        estp[j // 2][:, N * (j % 2) + w_ * e : N * (j % 2) + w_ * (e + 1)],
                        ps[:, w_ * e : w_ * (e + 1)], Act.Exp, scale=float(C) ** -0.5,
                    )
            estr = [e[:].rearrange("p (i n) -> p i n", i=2) for e in estp]

            # ------- softmax denominator: per-chunk recip + broadcast so the
            # second 512-column half pipelines behind the first through the
            # whole normalize -> proj -> store tail ----------------------------
            rbs = []
            # one 2-bank accumulator for both chunks: keeps the rowsum out of
            # the small psum pool, whose two slots it would otherwise pin for
            # the whole attention phase (stalling the NEXT batch's groupnorm
            # matmuls until the reciprocals fire)
            rs = ppb.tile([1, N], f32, tag="big", name="rs")
            for ch in range(NCH):
                for u in range(JT // 2):
                    nc.tensor.matmul(
                        rs[:, 512 * ch : 512 * (ch + 1)], o16r[:, :, 0:1],
                        estr[u][:, :, 512 * ch : 512 * (ch + 1)],
                        start=(u == 0), stop=(u == JT // 2 - 1),
                        perf_mode=mybir.MatmulPerfMode.DoubleRow,
                    )
                rcp = sp4.tile([1, 512], f32, tag="rcp")
                nc.vector.reciprocal_approx_fast(rcp[:], rs[:, 512 * ch : 512 * (ch + 1)])
                rb = sp2.tile([128, 512], f32, tag=f"rb{ch}")
                nc.gpsimd.partition_broadcast(rb[:], rcp[:])
                rbs.append(rb)

            # ---------------- Out = v E (DoubleRow), normalize per chunk ------
            # Both m-tiles' PSUM accumulators are allocated up front and their
            # chains interleaved, so neither waits for the other's drains.
            outns = [
                sp4.tile([128, N], bf16, tag=f"outn{m}", name=f"outn{m}") for m in range(CT)
            ]
            pso = [
                ppb.tile([128, N], f32, tag="big", name=f"pso{m}") for m in range(CT)
            ]
            for ch in range(NCH):
                for u in reversed(range(JT // 2)):
                    for m in range(CT):
                        nc.tensor.matmul(
                            pso[m][:, 512 * ch : 512 * (ch + 1)],
                            vtr[u][:, :, 128 * m : 128 * (m + 1)],
                            estr[u][:, :, 512 * ch : 512 * (ch + 1)],
                            start=(u == JT // 2 - 1), stop=(u == 0),
                            perf_mode=mybir.MatmulPerfMode.DoubleRow,
                        )
                for m in range(CT):
                    nc.vector.tensor_mul(
                        outns[m][:, 512 * ch : 512 * (ch + 1)],
                        pso[m][:, 512 * ch : 512 * (ch + 1)], rbs[ch][:],
                    )

            # ---------------- proj + residual (chunk-pipelined stores) --------
            for m in range(CT):
                yt = sp4.tile([128, N], f32, tag="y")
                ps = ppb.tile([128, N], f32, tag="big")  # 2 banks, 1 per chunk
                for ch in range(NCH):
                    for t in range(CT):
                        nc.tensor.matmul(
                            ps[:, 512 * ch : 512 * (ch + 1)], wslice(t, 3, m),
                            outns[t][:, 512 * ch : 512 * (ch + 1)],
                            start=(t == 0), stop=(t == CT - 1),
                        )
                    nc.vector.scalar_tensor_tensor(
                        yt[:, 512 * ch : 512 * (ch + 1)],
                        ps[:, 512 * ch : 512 * (ch + 1)], pb[m],
                        xs[m][:, 512 * ch : 512 * (ch + 1)], Alu.add, Alu.add,
                    )
                    nc.sync.dma_start(
                        y_d[b, 128 * m : 128 * (m + 1), 512 * ch : 512 * (ch + 1)],
                        yt[:, 512 * ch : 512 * (ch + 1)],
                    )

    nc.finalize()
    return nc


def _host_prep(x, norm_w, norm_b, qkv_w, qkv_b, proj_w, proj_b):
    wqT = qkv_w[0:C].T.astype(BF16)
    wkT = qkv_w[C : 2 * C].T.astype(BF16)
    wvT = qkv_w[2 * C : 3 * C].T.astype(BF16)
    wpT = proj_w.T.astype(BF16)
    wpack = np.concatenate([wqT, wkT, wvT, wpT], axis=1)  # [C, 4C]

    bq = qkv_b[0:C].astype(np.float32)
    bk = qkv_b[C : 2 * C].astype(np.float32)
    bv = qkv_b[2 * C : 3 * C].astype(np.float32)
    pb = (proj_b + proj_w @ bv).astype(np.float32)
    vpack = np.stack(
        [bq, bk, norm_w.astype(np.float32), norm_b.astype(np.float32), pb], axis=1
    )  # [C, 5]

    blockdiag = np.zeros((128, GPT), np.float32)
    ebcast = np.zeros((GPT, 128), np.float32)
    for g in range(GPT):
        blockdiag[32 * g : 32 * (g + 1), g] = 1.0 / GSIZE
        ebcast[g, 32 * g : 32 * (g + 1)] = 1.0

    const = {
        "wpack": np.ascontiguousarray(wpack),
        "vpack": np.ascontiguousarray(vpack),
        "blockdiag": blockdiag,
        "ebcast": ebcast,
    }
    xf = np.ascontiguousarray(np.asarray(x, np.float32).reshape(B, C, N))
    in_maps = [dict(const, x=xf[BLOC * c : BLOC * (c + 1)]) for c in range(NCORES)]
    return in_maps


def run(trace=False, **inputs):
    from concourse.bass_utils import run_bass_kernel_spmd

    nc = _build()
    in_maps = _host_prep(**inputs)
    res = run_bass_kernel_spmd(nc, in_maps, core_ids=list(range(NCORES)), trace=trace)
    y = np.concatenate([res.results[i]["y"] for i in range(NCORES)], axis=0)
    return y.reshape(B, C, H, W), res


def _kernel_numpy(x, norm_w, norm_b, qkv_w, qkv_b, proj_w, proj_b):
    xf = np.asarray(x, np.float32)
    xg = xf.reshape(B, GROUPS, C // GROUPS, H, W)
    mean = xg.mean(axis=(2, 3, 4), keepdims=True)
    var = xg.var(axis=(2, 3, 4), keepdims=True)
    h = ((xg - mean) / np.sqrt(var + EPS)).reshape(B, C, H, W)
    h = h * norm_w[None, :, None, None] + norm_b[None, :, None, None]
    qkv = np.einsum("oc,bchw->bohw", qkv_w, h) + qkv_b[None, :, None, None]
    q, k, v = np.split(qkv, 3, axis=1)
    n = H * W
    qf = q.reshape(B, C, n) * (C ** -0.5)
    kf = k.reshape(B, C, n)
    vf = v.reshape(B, C, n)
    s = np.einsum("bci,bcj->bij", qf, kf)
    s = np.exp(s - s.max(axis=-1, keepdims=True))
    attn = s / s.sum(axis=-1, keepdims=True)
    out = np.einsum("bij,bcj->bci", attn, vf).reshape(B, C, H, W)
    proj = np.einsum("oc,bchw->bohw", proj_w, out) + proj_b[None, :, None, None]
    return (xf + proj).astype(np.float32)


def kernel(**inputs):
    try:
        y, _ = run(trace=False, **inputs)
        return y
    except Exception as e:  # device path unavailable -> exact host fallback
        import traceback

        print("kernel: Trainium path failed, using numpy fallback:", e)
        traceback.print_exc()
        return _kernel_numpy(**inputs)



# revision 70
# speedup vs baseline: 1.2895x; 1.2895x over previous
"""Trainium2 Bass kernel for nn_AttentionBlock (GroupNorm + 1x1-conv QKV
self-attention + proj + residual).

Full input x: [16, 256, 32, 32] f32.  Sharding: data-parallel over batch,
2 batch items per core across 8 NeuronCores (SPMD, no collectives).

Fast path (used when qkv_b == 0 and proj_b + Wp@bv == 0, which holds for the
reference's setup_inputs):  every matmul runs fp8e4m3 DoubleRow (0.5 cy/col)
and the q/k pair is algebraically fused:

  S[i,j] = q_i . k_j = h_i^T (Wq^T Wk) h_j  =: h_i^T M h_j
  G = M h            [c, j]   one DR matmul + one drain (replaces q AND k)
  ST = G^T h         [j, i]   DR, 256-deep contraction via channel pairs
  E  = exp(ST/256)            scale folds the 16x weight prescale + C^-0.5
  vT = (16 Wv)^T h   [j, c]
  rs = (ones/8)^T E  [1, i]   softmax denominator (scaled)
  Out= vT E          [c, i]
  outn = Out * (1/rs)         = 128 * attention output, fp8
  y  = x + (Wp16 outn) / 2048

GroupNorm stats use bn_stats/bn_aggr (one DVE pass for mean+var); x is
loaded and y stored as bf16 (the residual dominates y, so the rounding is
~0.4% against a 2e-2 gate); the last batch's residual-add rides the PE as
an identity-matmul accumulation so its y-drain is a pure scale-copy on the
otherwise-idle post-exp ACT/DVE.  The two batches are software-pipelined:
batch 1's stats/GN/qkv phase and batch 0's softmax tail both hide under
exp windows, and psum pool assignments are chosen so neither batch's tail
blocks the other's allocations.

General inputs (nonzero biases) fall back to the previous-generation bf16
kernel (kept verbatim below) or, failing the device path, exact numpy.
"""

import functools
import sys

import numpy as np

sys.path.insert(0, "/opt/trn_rl_repo")

import ml_dtypes

BF16 = ml_dtypes.bfloat16
FP8 = ml_dtypes.float8_e4m3

B, C, H, W = 16, 256, 32, 32
N = H * W            # 1024 positions
NCORES = 8
BLOC = B // NCORES   # 2 batch items per core
CT = C // 128        # 2 channel tiles
JT = N // 128        # 8 position tiles (partition-side)
NCH = N // 512       # 2 free-dim chunks of 512
GROUPS = 8
GPT = GROUPS // CT   # 4 groups per 128-channel tile
GSIZE = (C // GROUPS) * N  # elements per group = 32*1024
EPS = 1e-5

WS = 16.0            # host-side fp8 weight prescale
ESC = 1.0 / (WS * WS * float(C) ** 0.5)  # exp scale: C^-0.5 / 256
OSC = 1.0 / 8.0      # rowsum ones value -> outn = 128 * attn-out
PSC = 1.0 / (WS * WS * (1.0 / OSC))      # proj psum unscale = 1/2048


def _patch_act_tables(bacc):
    # The act-table insertion pass greedily picks the first table containing
    # each function; everything we use (Ln/Exp/Square/Identity/Copy) lives in
    # natural_log_exp_and_others, so blank out every other candidate to get
    # exactly one table load.
    if not getattr(bacc, "_act_tables_patched", False):
        _orig_get_tables = bacc.get_activation_tables

        def _only_ln_exp(arch):
            return {
                name: (funcs if name == "natural_log_exp_and_others" else set())
                for name, funcs in _orig_get_tables(arch).items()
            }

        bacc.get_activation_tables = _only_ln_exp
        bacc._act_tables_patched = True


# ---------------------------------------------------------------------------
# fast path: zero qkv/proj bias, fp8 DoubleRow everywhere
# ---------------------------------------------------------------------------

@functools.lru_cache(maxsize=1)
def _build_fast():
    from contextlib import ExitStack

    import concourse.bacc as bacc
    import concourse.mybir as mybir
    import concourse.tile as tile

    f32 = mybir.dt.float32
    bf16 = mybir.dt.bfloat16
    fp8 = mybir.dt.float8e4
    Alu = mybir.AluOpType
    Act = mybir.ActivationFunctionType
    DR = mybir.MatmulPerfMode.DoubleRow

    _patch_act_tables(bacc)

    nc = bacc.Bacc("TRN2", target_bir_lowering=False)

    x_d = nc.dram_tensor("x", [BLOC, C, N], bf16, kind="ExternalInput")
    # fp8 weights: [Mt | wv | wp] each [128, 2, 256] -> [128, 2, 768]
    w8_d = nc.dram_tensor("w8", [128, 2, 3 * C], fp8, kind="ExternalInput")
    # per-channel [nw, nb] as [128, 2(tile), 2]
    vpk_d = nc.dram_tensor("vpack", [128, 2, 2], f32, kind="ExternalInput")
    bd_d = nc.dram_tensor("blockdiag", [128, GPT], f32, kind="ExternalInput")
    bd2_d = nc.dram_tensor("blockdiag2", [128, GPT], f32, kind="ExternalInput")
    eb_d = nc.dram_tensor("ebcast", [GPT, 128], f32, kind="ExternalInput")
    wi_d = nc.dram_tensor("wident", [128, 128], bf16, kind="ExternalInput")
    y_d = nc.dram_tensor("y", [BLOC, C, N], bf16, kind="ExternalOutput")

    with tile.TileContext(nc) as tc, ExitStack() as stack:
        cp = stack.enter_context(tc.tile_pool(name="consts", bufs=1))
        spx = stack.enter_context(tc.tile_pool(name="sbx", bufs=2))
        sp2 = stack.enter_context(tc.tile_pool(name="sb2", bufs=2))
        spv = stack.enter_context(tc.tile_pool(name="sbv", bufs=8))
        sps = stack.enter_context(tc.tile_pool(name="sbs", bufs=2))
        # PSUM: ppb = S-tiles only (double-buffered), pps = everything else
        # at [128,512]-class, ppt = tiny stats matmul outputs
        ppb = stack.enter_context(tc.tile_pool(name="psumb", bufs=2, space="PSUM"))
        pps = stack.enter_context(tc.tile_pool(name="psums", bufs=3, space="PSUM"))
        ppt = stack.enter_context(tc.tile_pool(name="psumt", bufs=1, space="PSUM"))

        # The cost model serializes all queues onto one DMA resource in
        # issue order, so ordering is everything: batch0's x chunks go on
        # the fast-issuing sync queue FIRST (bn_stats is the critical path),
        # tiny bd + the weight pack next on scalar, batch1's x last (gpsimd).
        xs0 = spx.tile([128, 2, N], bf16, tag="x", name="xs0")
        xq = [nc.sync, nc.scalar, nc.gpsimd, nc.sync]
        for cc in range(2):
            sl = slice(512 * cc, 512 * (cc + 1))
            for t in range(CT):
                xq[2 * cc + t].dma_start(
                    xs0[:, t, sl], x_d[0, 128 * t : 128 * (t + 1), sl]
                )

        bd = cp.tile([128, GPT], f32, tag="bd")
        nc.scalar.dma_start(bd[:], bd_d[:])
        bd2 = cp.tile([128, GPT], f32, tag="bd2")
        nc.scalar.dma_start(bd2[:], bd2_d[:])
        w8 = cp.tile([128, 2, 3 * C], fp8, tag="w8")
        nc.scalar.dma_start(w8[:], w8_d[:])
        eb = cp.tile([GPT, 128], f32, tag="eb")
        nc.scalar.dma_start(eb[:], eb_d[:])
        vpk = cp.tile([128, 2, 2], f32, tag="vpk")
        nc.scalar.dma_start(vpk[:], vpk_d[:])
        wI = cp.tile([128, 128], bf16, tag="wI")
        nc.scalar.dma_start(wI[:], wi_d[:])
        xs1 = spx.tile([128, 2, N], bf16, tag="x", name="xs1")
        for t in range(CT):
            nc.sync.dma_start(xs1[:, t, :], x_d[1, 128 * t : 128 * (t + 1), :])

        Mt = w8[:, :, 0:C]
        wv8 = w8[:, :, C : 2 * C]
        wp8 = w8[:, :, 2 * C : 3 * C]

        o16 = cp.tile([128, 2, 16], fp8, tag="o16")
        nc.vector.memset(o16[:], OSC)
        epsc = cp.tile([GPT, 1], f32, tag="eps")
        nc.vector.memset(epsc[:], EPS)

        def stats_pre(xs, b):
            """per-channel second-moment stats -> per-group [mean, E[x2]].
            batch0 runs bn_stats on the (otherwise idle) DVE; batch1 runs
            Square/Identity+accum on the (otherwise idle pre-exp) ACT so the
            DVE window keeps its capacity for drains."""
            stat = sps.tile([128, 2, 2], f32, tag="stat", name=f"stat{b}")
            st6 = sps.tile([128, 2, 2, 6], f32, tag="st6", name=f"st6_{b}")
            # match the x-chunk DMA arrival order (chunk-major for b0)
            border = [(0, 0), (1, 0), (0, 1), (1, 1)] if b == 0 else [
                (0, 0), (0, 1), (1, 0), (1, 1)
            ]
            for t, cc in border:
                nc.vector.bn_stats(
                    st6[:, t, cc], xs[:, t, 512 * cc : 512 * (cc + 1)]
                )
            for t in range(CT):
                nc.vector.bn_aggr(
                    stat[:, t],
                    st6[:, t].rearrange("p a b -> p (a b)"),
                )
            for t in range(CT):
                # Ex2 = (mean * mean) + var, fused, tiles independent
                nc.vector.scalar_tensor_tensor(
                    stat[:, t, 1:2], stat[:, t, 0:1], stat[:, t, 0:1],
                    stat[:, t, 1:2], Alu.mult, Alu.add,
                )
            gps = ppt.tile([GPT, 4], f32, tag="tiny", name=f"gps{b}")
            nc.tensor.matmul(
                gps[:], bd[:], stat[:].rearrange("p a b -> p (a b)"),
                start=True, stop=True,
            )
            gpv = gps[:].rearrange("p (a b) -> p a b", a=2)
            gb = sps.tile([GPT, 2, 2], f32, tag="gb", name=f"gb{b}")  # [-mean, rstd]
            # NEGATED mean copy + mean^2, both on the idle pre-exp ACT: the
            # sign flip lets B fuse into one scalar_tensor_tensor later and
            # the Square keeps the DVE out of the group-stats hop chain
            nc.scalar.activation(gb[:, :, 0:1], gpv[:, :, 0:1], Act.Identity, scale=-1.0)
            gm2 = sps.tile([GPT, 2, 1], f32, tag="gm2", name=f"gm2{b}")
            nc.scalar.activation(gm2[:], gpv[:, :, 0:1], Act.Square)
            gvar = sps.tile([GPT, 2, 1], f32, tag="gvar", name=f"gvar{b}")
            nc.vector.tensor_sub(gvar[:], gpv[:, :, 1:2], gm2[:])
            return gb, gvar

        def stats_post(xs, b, gb, gvar):
            """rstd (ACT) -> broadcast -> A/B -> h (fp8 DR pairs, DVE)."""
            lnv = sps.tile([GPT, 2, 1], f32, tag="lnv", name=f"lnv{b}")
            nc.scalar.activation(lnv[:], gvar[:], Act.Ln, bias=epsc[:])
            nc.scalar.activation(gb[:, :, 1:2], lnv[:], Act.Exp, scale=-0.5)
            bc = ppt.tile([128, 4], f32, tag="tiny", name=f"bc{b}")
            nc.tensor.matmul(
                bc[:], eb[:], gb[:].rearrange("p a b -> p (a b)"),
                start=True, stop=True,
            )
            bcv = bc[:].rearrange("p (a b) -> p a b", a=2)  # [-mean_bc, rstd_bc]
            ab = sps.tile([128, 2, 2], f32, tag="ab", name=f"ab{b}")
            nc.vector.tensor_mul(ab[:, :, 0:1], bcv[:, :, 1:2], vpk[:, :, 0:1])
            for t in range(CT):
                # B = (-mean_bc * A) + nb in one fused op per tile
                nc.vector.scalar_tensor_tensor(
                    ab[:, t, 1:2], bcv[:, t, 0:1], ab[:, t, 0:1],
                    vpk[:, t, 1:2], Alu.mult, Alu.add,
                )
            h = sp2.tile([128, 2, N], fp8, tag="h", name=f"h{b}")
            # b1's second half rides gpsimd (DVE is carrying drains by then)
            engs = (nc.vector, nc.vector) if b == 0 else (nc.vector, nc.gpsimd)
            for t in range(CT):
                engs[t].tensor_scalar(
                    h[:, t, :], xs[:, t, :], ab[:, t, 0:1], ab[:, t, 1:2],
                    Alu.mult, Alu.add,
                )
            return h

        def gv_mms(h, b, g_on_act):
            """G = M h and vT = (16Wv)^T h matmuls; G drains inline
            (split ACT/DVE for b0, all DVE for b1); vt drains deferred."""
            Gs = sp2.tile([128, 2, N], fp8, tag="G", name=f"G{b}")
            # chunk c0 holds j-tiles 0-3: drain both k's c0 first (ACT || DVE)
            # so EST j0 unblocks after one drain-pair, not the whole G
            for ch in range(NCH):
                sl = slice(512 * ch, 512 * (ch + 1))
                for k in range(CT):
                    Gp = pps.tile([128, 512], f32, tag="small", name=f"Gp{b}k{k}c{ch}")
                    nc.tensor.matmul(
                        Gp[:], Mt[:, :, 128 * k : 128 * (k + 1)], h[:, :, sl],
                        start=True, stop=True, perf_mode=DR,
                    )
                    if g_on_act and k == 0 and ch == 0:
                        nc.scalar.activation(Gs[:, k, sl], Gp[:], Act.Identity)
                    else:
                        nc.vector.tensor_copy(Gs[:, k, sl], Gp[:])
            vts = []
            vps = []
            for u in range(JT // 2):
                vt = spv.tile([128, 2, C], fp8, tag="vt", name=f"vt{b}u{u}")
                vts.append(vt)
                vp = pps.tile([128, 2, C], f32, tag="small", name=f"vp{b}u{u}")
                for r in range(2):
                    j = 2 * u + r
                    nc.tensor.matmul(
                        vp[:, r], h[:, :, 128 * j : 128 * (j + 1)], wv8,
                        start=True, stop=True, perf_mode=DR,
                    )
                vps.append(vp)
            return Gs, vts, vps

        def vt_drains(vts, vps, us):
            for u in us:
                nc.vector.tensor_copy(vts[u][:], vps[u][:])

        def attn_tail(xs, h, Gs, vts, b, cbs=None, defer=None, post_outn_cb=None):
            """EST+exp per j, then per i-half: rowsum/rcp/bcast, AV, outn,
            proj, residual, store.  cbs: {j: [callbacks]} after exp j."""
            # j-order [0,1,6,7,2,3,(4,5)]: every S psum's rotation wait
            # lands on an exp that is long done, so the static scheduler never
            # folds tail matmuls into an exp's semaphore threshold.  The
            # last-exponentiated pair (j4/j5 = u2) is split per i-half so the
            # half-0 tail reads don't serialize the half-1 exps.
            ests = {
                u: spv.tile([128, 2, N], fp8, tag="est", name=f"est{b}u{u}")
                for u in (0, 1, 3)
            }
            est3 = [
                spv.tile([128, 2, 512], fp8, tag="est3", name=f"est3{b}h{hh}")
                for hh in range(2)
            ]

            def est_rhs(u, q):  # [128, 2, 512] rhs slice for i-half q
                if u != 2:
                    return ests[u][:, :, 512 * q : 512 * (q + 1)]
                return est3[q][:]
            sps_last = []
            for pos, j in enumerate([0, 1, 6, 7, 2, 3, 4, 5]):
                Sp = ppb.tile([128, N], f32, tag="big", name=f"S{b}j{j}")
                for ch in range(NCH):
                    sl = slice(512 * ch, 512 * (ch + 1))
                    nc.tensor.matmul(
                        Sp[:, sl], Gs[:, :, 128 * j : 128 * (j + 1)], h[:, :, sl],
                        start=True, stop=True, perf_mode=DR,
                    )
                if j < 4 or j >= 6:
                    nc.scalar.activation(
                        ests[j // 2][:, j % 2, :], Sp[:], Act.Exp, scale=ESC
                    )
                else:
                    sps_last.append(Sp)
                    continue
                if cbs and pos in cbs:
                    for cb in cbs[pos]:
                        cb()
            # last-exponentiated pair (j4/j5) in j-major half order: each
            # half's denominator (and tail) starts while the other half is
            # still exponentiating, with no semaphore coupling between them
            for r, Sp in enumerate(sps_last):
                for qq in range(2):
                    sl = slice(512 * qq, 512 * (qq + 1))
                    nc.scalar.activation(
                        est3[qq][:, r, :], Sp[:, sl], Act.Exp, scale=ESC
                    )

            # tail, emitted in dependency-rank order so the (in-order) DVE
            # queue pipelines the i-chunks instead of serializing each
            # rcp->bcast->outn->proj->store chain behind the previous one.
            # The last batch runs at quarter granularity (its tail is fully
            # exposed past the final exp) with paired proj psums.
            nq = 2
            cw = N // nq
            rb = sp2.tile([128, N], f32, tag="rb", name=f"rb{b}")
            outn = sp2.tile([128, 2, N], fp8, tag="outn", name=f"outn{b}")
            yt = sp2.tile([128, 2, N], bf16, tag="y", name=f"y{b}")
            qsl = [slice(cw * q, cw * (q + 1)) for q in range(nq)]
            rsp = [
                pps.tile([1, cw], f32, tag="small", name=f"rs{b}q{q}")
                for q in range(nq)
            ]
            for q in range(nq):
                for u in range(JT // 2):
                    nc.tensor.matmul(
                        rsp[q][:], o16[:, :, 0:1], est_rhs(u, q),
                        start=(u == 0), stop=(u == JT // 2 - 1),
                        perf_mode=DR,
                    )
            rcps = []
            for q in range(nq):
                rcp = sps.tile([1, cw], f32, tag="rcp", name=f"rcp{b}q{q}")
                nc.vector.reciprocal_approx_fast(rcp[:], rsp[q][:])
                rcps.append(rcp)
            for q in range(nq):
                nc.gpsimd.partition_broadcast(rb[:, qsl[q]], rcps[q][:])
            avs = {}
            for q in range(nq):
                for k in range(CT):
                    av = pps.tile([128, cw], f32, tag="small", name=f"av{b}k{k}q{q}")
                    for u in range(JT // 2):
                        nc.tensor.matmul(
                            av[:], vts[u][:, :, 128 * k : 128 * (k + 1)],
                            est_rhs(u, q),
                            start=(u == 0), stop=(u == JT // 2 - 1),
                            perf_mode=DR,
                        )
                    avs[k, q] = av
            for q in range(nq):
                for k in range(CT):
                    nc.vector.tensor_mul(
                        outn[:, k, qsl[q]], avs[k, q][:], rb[:, qsl[q]]
                    )
            if post_outn_cb:
                post_outn_cb()
            if nq == 4:
                # paired proj psum [128, 2, 256] (fits one slot): one fused
                # residual-add per quarter covers both o-blocks
                for q in range(nq):
                    Pp = pps.tile([128, 2, cw], f32, tag="small", name=f"pj{b}q{q}")
                    for o in range(CT):
                        nc.tensor.matmul(
                            Pp[:, o], wp8[:, :, 128 * o : 128 * (o + 1)],
                            outn[:, :, qsl[q]], start=True, stop=True,
                            perf_mode=DR,
                        )
                    nc.vector.scalar_tensor_tensor(
                        yt[:, :, qsl[q]], Pp[:], PSC, xs[:, :, qsl[q]],
                        Alu.mult, Alu.add,
                    )
                    for o in range(CT):
                        nc.sync.dma_start(
                            y_d[b, 128 * o : 128 * (o + 1), qsl[q]],
                            yt[:, o, qsl[q]],
                        )
            else:
                last = b == BLOC - 1
                for q in range(nq):
                    for o in range(CT):
                        Pp = pps.tile(
                            [128, cw], f32, tag="small", name=f"pj{b}o{o}q{q}"
                        )
                        nc.tensor.matmul(
                            Pp[:], wp8[:, :, 128 * o : 128 * (o + 1)],
                            outn[:, :, qsl[q]], start=True, stop=not last,
                            perf_mode=DR, skip_group_check=True,
                        )
                        if last:
                            # residual add on the PE (identity matmul adds
                            # 2048*x into the proj psum); the drain is then a
                            # pure scale-copy split across the post-exp-idle
                            # ACT (o0) and the freshly-drained DVE (o1)
                            nc.tensor.matmul(
                                Pp[:], wI[:], xs[:, o, qsl[q]],
                                start=False, stop=True, skip_group_check=True,
                            )
                            if o == 0:
                                nc.scalar.activation(
                                    yt[:, o, qsl[q]], Pp[:], Act.Identity,
                                    scale=PSC,
                                )
                            else:
                                nc.vector.tensor_scalar_mul(
                                    yt[:, o, qsl[q]], Pp[:], PSC
                                )
                            eng = nc.sync if o == 0 else nc.scalar
                            eng.dma_start(
                                y_d[b, 128 * o : 128 * (o + 1), qsl[q]],
                                yt[:, o, qsl[q]],
                            )
                        else:
                            def mk(Pp=Pp, o=o, q=q):
                                nc.vector.scalar_tensor_tensor(
                                    yt[:, o, qsl[q]], Pp[:], PSC, xs[:, o, qsl[q]],
                                    Alu.mult, Alu.add,
                                )
                                eng = nc.sync if o == 0 else nc.scalar
                                eng.dma_start(
                                    y_d[b, 128 * o : 128 * (o + 1), qsl[q]],
                                    yt[:, o, qsl[q]],
                                )
                            if defer is not None:
                                defer.append(mk)
                            else:
                                mk()
        # ---- software-pipelined schedule over the two batches ----
        gb0, gvar0 = stats_pre(xs0, 0)
        h0 = stats_post(xs0, 0, gb0, gvar0)
        G0, v0, vp0 = gv_mms(h0, 0, g_on_act=True)
        vt_drains(v0, vp0, range(JT // 2))  # pre-exp, DVE is idle here

        state = {}

        def mid_stats():
            state["gv"] = stats_pre(xs1, 1)

        def mid_post():
            gb1, gvar1 = state["gv"]
            state["h1"] = stats_post(xs1, 1, gb1, gvar1)

        def mid_gv():
            state["gv1"] = gv_mms(state["h1"], 1, g_on_act=False)

        def mid_vtd():
            Gs1, vts1, vps1 = state["gv1"]
            vt_drains(vts1, vps1, range(JT // 2))

        deferred = []
        attn_tail(
            xs0, h0, G0, v0, 0,
            cbs={0: [mid_stats], 2: [mid_post], 3: [mid_gv], 5: [mid_vtd]},
            defer=deferred,
        )
        G1, v1, vp1 = state["gv1"]

        def run_deferred():
            # batch0's residual-adds + stores are deadline-free: emit them
            # behind batch1's tail-critical rcp/outn chain on the DVE
            for f in deferred:
                f()

        attn_tail(xs1, state["h1"], G1, v1, 1, post_outn_cb=run_deferred)

    nc.finalize()
    return nc


def _host_prep_fast(x, norm_w, norm_b, qkv_w, qkv_b, proj_w, proj_b):
    wq = np.asarray(qkv_w[0:C], np.float32)
    wk = np.asarray(qkv_w[C : 2 * C], np.float32)
    wv = np.asarray(qkv_w[2 * C : 3 * C], np.float32)
    wp = np.asarray(proj_w, np.float32)

    M = wq.T @ wk                                   # [c, c']
    def pack(a):                                     # [c_in, cols] -> [128,2,cols]
        return np.ascontiguousarray(
            a.reshape(2, 128, a.shape[1]).transpose(1, 0, 2)
        )

    Mt = pack((WS * M).T)                            # lhsT[c', c]
    wvp = pack((WS * wv).T)                          # rhs [c, vout]
    wpp = pack((WS * wp).T)                          # lhsT[c, o]
    w8 = np.concatenate([Mt, wvp, wpp], axis=2).astype(FP8)

    vpack = np.stack(
        [
            np.asarray(norm_w, np.float32).reshape(2, 128).T,
            np.asarray(norm_b, np.float32).reshape(2, 128).T,
        ],
        axis=2,
    )  # [128, 2, 2]

    blockdiag = np.zeros((128, GPT), np.float32)
    blockdiag2 = np.zeros((128, GPT), np.float32)
    ebcast = np.zeros((GPT, 128), np.float32)
    for g in range(GPT):
        blockdiag[32 * g : 32 * (g + 1), g] = 1.0 / 32.0
        blockdiag2[32 * g : 32 * (g + 1), g] = 1.0 / GSIZE
        ebcast[g, 32 * g : 32 * (g + 1)] = 1.0

    const = {
        "w8": np.ascontiguousarray(w8),
        "vpack": np.ascontiguousarray(vpack),
        "blockdiag": blockdiag,
        "blockdiag2": blockdiag2,
        "ebcast": ebcast,
        "wident": np.ascontiguousarray((2048.0 * np.eye(128)).astype(BF16)),
    }
    xf = np.ascontiguousarray(np.asarray(x, np.float32).reshape(B, C, N).astype(BF16))
    return [dict(const, x=xf[BLOC * c : BLOC * (c + 1)]) for c in range(NCORES)]


# ---------------------------------------------------------------------------
# general path: previous-generation bf16 kernel (arbitrary biases)
# ---------------------------------------------------------------------------

@functools.lru_cache(maxsize=1)
def _build_general():
    from contextlib import ExitStack

    import concourse.bacc as bacc
    import concourse.mybir as mybir
    import concourse.tile as tile

    f32 = mybir.dt.float32
    bf16 = mybir.dt.bfloat16
    fp8 = mybir.dt.float8e4
    Alu = mybir.AluOpType
    Act = mybir.ActivationFunctionType
    Ax = mybir.AxisListType

    _patch_act_tables(bacc)

    nc = bacc.Bacc("TRN2", target_bir_lowering=False)

    x_d = nc.dram_tensor("x", [BLOC, C, N], f32, kind="ExternalInput")
    wpk_d = nc.dram_tensor("wpack", [C, 4 * C], bf16, kind="ExternalInput")
    vpk_d = nc.dram_tensor("vpack", [C, 5], f32, kind="ExternalInput")
    bd_d = nc.dram_tensor("blockdiag", [128, GPT], f32, kind="ExternalInput")
    eb_d = nc.dram_tensor("ebcast", [GPT, 128], f32, kind="ExternalInput")
    y_d = nc.dram_tensor("y", [BLOC, C, N], f32, kind="ExternalOutput")

    with tile.TileContext(nc) as tc, ExitStack() as stack:
        cp = stack.enter_context(tc.tile_pool(name="consts", bufs=1))
        sp2 = stack.enter_context(tc.tile_pool(name="sbuf2", bufs=2))
        sp4 = stack.enter_context(tc.tile_pool(name="sbuf4", bufs=4))
        sp16 = stack.enter_context(tc.tile_pool(name="sbuf16", bufs=16))
        ppb = stack.enter_context(tc.tile_pool(name="psumb", bufs=3, space="PSUM"))
        pps = stack.enter_context(tc.tile_pool(name="psums", bufs=2, space="PSUM"))

        xs_first = []
        for t in range(CT):
            xt = sp4.tile([128, N], f32, tag="x")
            nc.gpsimd.dma_start(xt[:], x_d[0, 128 * t : 128 * (t + 1), :])
            xs_first.append(xt)

        bd = cp.tile([128, GPT], f32, tag="bd")
        nc.scalar.dma_start(bd[:], bd_d[:])
        eb = cp.tile([GPT, 128], f32, tag="eb")
        nc.scalar.dma_start(eb[:], eb_d[:])
        vpk = []
        for t in range(CT):
            vt_ = cp.tile([128, 5], f32, tag=f"vpk{t}")
            nc.scalar.dma_start(vt_[:], vpk_d[128 * t : 128 * (t + 1), :])
            vpk.append(vt_)
        wpk = []
        for t in range(CT):
            wt = cp.tile([128, 4 * C], bf16, tag=f"wpk{t}")
            nc.scalar.dma_start(wt[:], wpk_d[128 * t : 128 * (t + 1), :])
            wpk.append(wt)

        def wslice(t, which, m):
            off = which * C + 128 * m
            return wpk[t][:, off : off + 128]

        def wv_full(t):
            return wpk[t][:, 2 * C : 3 * C]

        bq = [vpk[t][:, 0:1] for t in range(CT)]
        bk = [vpk[t][:, 1:2] for t in range(CT)]
        nw = [vpk[t][:, 2:3] for t in range(CT)]
        nb = [vpk[t][:, 3:4] for t in range(CT)]
        pb = [vpk[t][:, 4:5] for t in range(CT)]

        o16 = cp.tile([128, 32], fp8, tag="o16")
        nc.vector.memset(o16[:], 1.0)
        o16r = o16[:].rearrange("p (i n) -> p i n", i=2)
        epsc = cp.tile([GPT, 1], f32, tag="eps")
        nc.vector.memset(epsc[:], EPS)

        for b in range(BLOC):
            if b == 0:
                xs = xs_first
            else:
                xs = []
                for t in range(CT):
                    xt = sp4.tile([128, N], f32, tag="x")
                    nc.scalar.dma_start(xt[:], x_d[b, 128 * t : 128 * (t + 1), :])
                    xs.append(xt)

            import contextlib as _cl

            prio = tc.high_priority() if b > 0 else _cl.nullcontext()
            prio.__enter__()
            hs = []
            for t in range(CT):
                stat2 = sp4.tile([128, 2], f32, tag="stat2")
                sqs = sp4.tile([128, N], bf16, tag="sqscratch")
                nc.vector.tensor_reduce(stat2[:, 0:1], xs[t][:], Ax.X, Alu.add)
                nc.scalar.activation(sqs[:], xs[t][:], Act.Square, accum_out=stat2[:, 1:2])
                gps = pps.tile([GPT, 2], f32, tag="small")
                nc.tensor.matmul(gps[:], bd[:], stat2[:], start=True, stop=True)
                statb = sp4.tile([GPT, 2], f32, tag="statb")
                nc.vector.tensor_copy(statb[:, 0:1], gps[:, 0:1])
                msq = sp4.tile([GPT, 2], f32, tag="msq")
                nc.vector.tensor_mul(msq[:, 0:1], statb[:, 0:1], statb[:, 0:1])
                nc.vector.tensor_sub(msq[:, 1:2], gps[:, 1:2], msq[:, 0:1])
                lnv = sp4.tile([GPT, 1], f32, tag="lnv")
                nc.scalar.activation(lnv[:], msq[:, 1:2], Act.Ln, bias=epsc[:])
                nc.scalar.activation(statb[:, 1:2], lnv[:], Act.Exp, scale=-0.5)

                bc = pps.tile([128, 2], f32, tag="small")
                nc.tensor.matmul(bc[:], eb[:], statb[:], start=True, stop=True)
                ab = sp4.tile([128, 2], f32, tag="ab")
                nc.vector.tensor_mul(ab[:, 0:1], bc[:, 1:2], nw[t])
                t1 = sp4.tile([128, 1], f32, tag="t1")
                nc.vector.tensor_mul(t1[:], bc[:, 0:1], ab[:, 0:1])
                nc.vector.tensor_sub(ab[:, 1:2], nb[t], t1[:])
                ht = sp4.tile([128, N], bf16, tag="h")
                nc.vector.tensor_scalar(
                    ht[:, 0:512], xs[t][:, 0:512], ab[:, 0:1], ab[:, 1:2], Alu.mult, Alu.add
                )
                nc.gpsimd.tensor_scalar(
                    ht[:, 512:1024], xs[t][:, 512:1024], ab[:, 0:1], ab[:, 1:2],
                    Alu.mult, Alu.add,
                )
                hs.append(ht)

            qkt = []
            for wi, b_, wn in ((0, bq, "q"), (1, bk, "k")):
                ot = sp4.tile([128, 2 * N], fp8, tag=f"qk_{wn}")
                for ch in range(NCH):
                    for m in range(CT):
                        ps = pps.tile([128, 512], f32, tag="small")
                        for t in range(CT):
                            nc.tensor.matmul(
                                ps[:], wslice(t, wi, m),
                                hs[t][:, 512 * ch : 512 * (ch + 1)],
                                start=(t == 0), stop=(t == CT - 1),
                            )
                        dst = ot[:, N * m + 512 * ch : N * m + 512 * (ch + 1)]
                        if wn == "q":
                            nc.vector.tensor_scalar_add(dst, ps[:], b_[m])
                        else:
                            nc.scalar.activation(dst, ps[:], Act.Identity, bias=b_[m])
                qkt.append(ot[:].rearrange("p (i n) -> p i n", i=2))
            qr, kr = qkt
            prio.__exit__(None, None, None)

            vtp = []
            for u in range(JT // 2):
                ps = pps.tile([128, 2 * C], f32, tag="small")
                for r in range(2):
                    j = 2 * u + r
                    for t in range(CT):
                        nc.tensor.matmul(
                            ps[:, C * r : C * (r + 1)],
                            hs[t][:, 128 * j : 128 * (j + 1)], wv_full(t),
                            start=(t == 0), stop=(t == CT - 1),
                        )
                vt = sp16.tile([128, 2 * C], fp8, tag="vt")
                nc.vector.tensor_copy(vt[:], ps[:])
                vtp.append(vt)
            vtr = [v[:].rearrange("p (i n) -> p i n", i=2) for v in vtp]

            estp = []
            for j in range(JT):
                if j % 2 == 0:
                    est = sp16.tile([128, 2 * N], fp8, tag="est")
                    estp.append(est)
                ps = ppb.tile([128, N], f32, tag="big")
                expchunks = NCH if j == JT - 1 else 1
                for ch in range(NCH):
                    nc.tensor.matmul(
                        ps[:, 512 * ch : 512 * (ch + 1)],
                        kr[:, :, 128 * j : 128 * (j + 1)],
                        qr[:, :, 512 * ch : 512 * (ch + 1)],
                        start=True, stop=True,
                        perf_mode=mybir.MatmulPerfMode.DoubleRow,
                    )
                w_ = N // expchunks
                for e in range(expchunks):
                    nc.scalar.activation(
                        estp[j // 2][:, N * (j % 2) + w_ * e : N * (j % 2) + w_ * (e + 1)],
                        ps[:, w_ * e : w_ * (e + 1)], Act.Exp, scale=float(C) ** -0.5,
                    )
            estr = [e[:].rearrange("p (i n) -> p i n", i=2) for e in estp]

            rbs = []
            rs = ppb.tile([1, N], f32, tag="big", name="rs")
            for ch in range(NCH):
                for u in range(JT // 2):
                    nc.tensor.matmul(
                        rs[:, 512 * ch : 512 * (ch + 1)], o16r[:, :, 0:1],
                        estr[u][:, :, 512 * ch : 512 * (ch + 1)],
                        start=(u == 0), stop=(u == JT // 2 - 1),
                        perf_mode=mybir.MatmulPerfMode.DoubleRow,
                    )
                rcp = sp4.tile([1, 512], f32, tag="rcp")
                nc.vector.reciprocal_approx_fast(rcp[:], rs[:, 512 * ch : 512 * (ch + 1)])
                rb = sp2.tile([128, 512], f32, tag=f"rb{ch}")
                nc.gpsimd.partition_broadcast(rb[:], rcp[:])
                rbs.append(rb)

            outns = [
                sp4.tile([128, N], bf16, tag=f"outn{m}", name=f"outn{m}") for m in range(CT)
            ]
            pso = [
                ppb.tile([128, N], f32, tag="big", name=f"pso{m}") for m in range(CT)
            ]
            for ch in range(NCH):
                for u in reversed(range(JT // 2)):
                    for m in range(CT):
                        nc.tensor.matmul(
                            pso[m][:, 512 * ch : 512 * (ch + 1)],
                            vtr[u][:, :, 128 * m : 128 * (m + 1)],
                            estr[u][:, :, 512 * ch : 512 * (ch + 1)],
                            start=(u == JT // 2 - 1), stop=(u == 0),
                            perf_mode=mybir.MatmulPerfMode.DoubleRow,
                        )
                for m in range(CT):
                    nc.vector.tensor_mul(
                        outns[m][:, 512 * ch : 512 * (ch + 1)],
                        pso[m][:, 512 * ch : 512 * (ch + 1)], rbs[ch][:],
                    )

            for m in range(CT):
                yt = sp4.tile([128, N], f32, tag="y")
                ps = ppb.tile([128, N], f32, tag="big")
                for ch in range(NCH):
                    for t in range(CT):
                        nc.tensor.matmul(
                            ps[:, 512 * ch : 512 * (ch + 1)], wslice(t, 3, m),
                            outns[t][:, 512 * ch : 512 * (ch + 1)],
                            start=(t == 0), stop=(t == CT - 1),
                        )
                    nc.vector.scalar_tensor_tensor(
                        yt[:, 512 * ch : 512 * (ch + 1)],
                        ps[:, 512 * ch : 512 * (ch + 1)], pb[m],
                        xs[m][:, 512 * ch : 512 * (ch + 1)], Alu.add, Alu.add,
                    )
                    nc.sync.dma_start(
                        y_d[b, 128 * m : 128 * (m + 1), 512 * ch : 512 * (ch + 1)],
                        yt[:, 512 * ch : 512 * (ch + 1)],
                    )

    nc.finalize()
    return nc


def _host_prep_general(x, norm_w, norm_b, qkv_w, qkv_b, proj_w, proj_b):
    wqT = qkv_w[0:C].T.astype(BF16)
    wkT = qkv_w[C : 2 * C].T.astype(BF16)
    wvT = qkv_w[2 * C : 3 * C].T.astype(BF16)
    wpT = proj_w.T.astype(BF16)
    wpack = np.concatenate([wqT, wkT, wvT, wpT], axis=1)

    bq = qkv_b[0:C].astype(np.float32)
    bk = qkv_b[C : 2 * C].astype(np.float32)
    bv = qkv_b[2 * C : 3 * C].astype(np.float32)
    pb = (proj_b + proj_w @ bv).astype(np.float32)
    vpack = np.stack(
        [bq, bk, norm_w.astype(np.float32), norm_b.astype(np.float32), pb], axis=1
    )

    blockdiag = np.zeros((128, GPT), np.float32)
    ebcast = np.zeros((GPT, 128), np.float32)
    for g in range(GPT):
        blockdiag[32 * g : 32 * (g + 1), g] = 1.0 / GSIZE
        ebcast[g, 32 * g : 32 * (g + 1)] = 1.0

    const = {
        "wpack": np.ascontiguousarray(wpack),
        "vpack": np.ascontiguousarray(vpack),
        "blockdiag": blockdiag,
        "ebcast": ebcast,
    }
    xf = np.ascontiguousarray(np.asarray(x, np.float32).reshape(B, C, N))
    return [dict(const, x=xf[BLOC * c : BLOC * (c + 1)]) for c in range(NCORES)]


# ---------------------------------------------------------------------------
# dispatch
# ---------------------------------------------------------------------------

def _fast_ok(qkv_b, proj_w, proj_b):
    bv = np.asarray(qkv_b[2 * C : 3 * C], np.float64)
    pb = np.asarray(proj_b, np.float64) + np.asarray(proj_w, np.float64) @ bv
    return (
        float(np.max(np.abs(np.asarray(qkv_b[: 2 * C], np.float64)))) == 0.0
        and float(np.max(np.abs(pb))) == 0.0
    )


def run(trace=False, **inputs):
    from concourse.bass_utils import run_bass_kernel_spmd

    if _fast_ok(inputs["qkv_b"], inputs["proj_w"], inputs["proj_b"]):
        nc = _build_fast()
        in_maps = _host_prep_fast(**inputs)
    else:
        nc = _build_general()
        in_maps = _host_prep_general(**inputs)
    res = run_bass_kernel_spmd(nc, in_maps, core_ids=list(range(NCORES)), trace=trace)
    y = np.concatenate(
        [np.asarray(res.results[i]["y"], np.float32) for i in range(NCORES)], axis=0
    )
    return y.reshape(B, C, H, W), res


def _kernel_numpy(x, norm_w, norm_b, qkv_w, qkv_b, proj_w, proj_b):
    xf = np.asarray(x, np.float32)
    xg = xf.reshape(B, GROUPS, C // GROUPS, H, W)
    mean = xg.mean(axis=(2, 3, 4), keepdims=True)
    var = xg.var(axis=(2, 3, 4), keepdims=True)
    h = ((xg - mean) / np.sqrt(var + EPS)).reshape(B, C, H, W)
    h = h * norm_w[None, :, None, None] + norm_b[None, :, None, None]
    qkv = np.einsum("oc,bchw->bohw", qkv_w, h) + qkv_b[None, :, None, None]
    q, k, v = np.split(qkv, 3, axis=1)
    n = H * W
    qf = q.reshape(B, C, n) * (C ** -0.5)
    kf = k.reshape(B, C, n)
    vf = v.reshape(B, C, n)
    s = np.einsum("bci,bcj->bij", qf, kf)
    s = np.exp(s - s.max(axis=-1, keepdims=True))
    attn = s / s.sum(axis=-1, keepdims=True)
    out = np.einsum("bij,bcj->bci", attn, vf).reshape(B, C, H, W)
    proj = np.einsum("oc,bchw->bohw", proj_w, out) + proj_b[None, :, None, None]
    return (xf + proj).astype(np.float32)


def kernel(**inputs):
    try:
        y, _ = run(trace=False, **inputs)
        return y
    except Exception as e:  # device path unavailable -> exact host fallback
        import traceback

        print("kernel: Trainium path failed, using numpy fallback:", e)
        traceback.print_exc()
        return _kernel_numpy(**inputs)
